# revision 11
# baseline (speedup 1.0000x reference)
"""Trainium2 Bass kernel for nn_DTFN_38405597561803 (gnn_message_passing).

Model (reference):
    h  = emb[x]                                   # [B,S,D] gather
    m  = softplus(h @ w_mass.T + b_mass) + EPS
    dt = sigmoid(cfl_raw)
    repeat K=3:
        hid = tanh(left @ w_f1_l.T + right @ w_f1_r.T + b_f1)   # left/right = adjacent positions
        F   = softplus(hid @ w_f2.T + b_f2)
        m   = max(m + dt * (F[i-1] - F[i]), EPS)                # conservative 1-D flux stencil
    out = m @ w_dec.T + b_dec                      # [B,S,V] decode, V=32000 (memory bound)

Sharding: sequence-parallel, 8 cores = 4 batches x 2 halves of S=2048.
Each core gets its 1024 owned positions plus a K=3 halo on each interior
side (built host-side by overlapping the shards -> no device-to-device
communication).  Chunk boundaries that are not true sequence boundaries
are handled exactly by masking the 3 outermost edge fluxes each step
(errors cannot cross a masked edge); true boundaries coincide with chunk
ends so the no-flux boundary condition is automatic.  dt is folded into
the edge mask.  Layout on device is feature-major (mT = [D, positions]) so
the stencil shift is a free-dim AP offset and all matmuls need no
transposes (only the initial embedding gather is transposed via the PE).

Decode: out[r, v] tiles of [128 rows, 500 vocab]; stationary = mT column
block, moving = resident w_dec.T slice (float32r matmuls, full PE rate);
b_dec added by an extra K=1 matmul with a ones vector accumulating into
the same PSUM bank; PSUM -> SBUF copies alternate Scalar/Vector engines;
SBUF -> HBM DMA writes 131 MB/core (the memory-roofline term).
"""

import sys

if "/opt/trn_rl_repo" not in sys.path:
    sys.path.insert(0, "/opt/trn_rl_repo")

import numpy as np

B, S, D, V, K = 4, 2048, 128, 32000, 3
EPS = 1e-6
NCORES = 8
HALO = K                      # 3
LOWN = S // 2                 # 1024 owned positions per core
L = LOWN + 2 * HALO           # 1030 local positions
NE = L - 1                    # 1029 local edges
NEP = NE + 1                  # 1030: padded even edge count (fp32r needs even free dims);
                              # the extra fake edge only ever pollutes the outermost halo
                              # position (1 col/step, 3 steps, owned ends 3 cols earlier)
NT = (L + 127) // 128         # 9 gather tiles (covers 1152 >= 1030)
LPAD = NT * 128               # 1152
VBLK = 500                    # vocab block (<= 512 PSUM bank, 64*500 = 32000)
NVB = V // VBLK               # 64
RT = LOWN // 128              # 8 row tiles per core
WDEC_CHUNK = 4000             # w_decT load granularity (8 chunks)

_CACHE = {}


def build_program():
    """Build (and bacc-compile) the single-core SPMD Bass program."""
    import concourse.bacc as bacc
    import concourse.bass as bass
    import concourse.tile as tile
    from concourse import mybir

    f32 = mybir.dt.float32
    f32r = mybir.dt.float32r
    i32 = mybir.dt.int32
    AF = mybir.ActivationFunctionType

    nc = bacc.Bacc(
        trn_type="TRN2",
        target_bir_lowering=False,
        debug=False,
        enable_asserts=False,
        num_devices=NCORES,
    )

    d_idx = nc.dram_tensor("idx", [128, NT], i32, kind="ExternalInput").ap()
    d_emb = nc.dram_tensor("emb", [V, D], f32, kind="ExternalInput").ap()
    d_wm = nc.dram_tensor("wmT", [D, D], f32r, kind="ExternalInput").ap()
    d_wl = nc.dram_tensor("wf1lT", [D, D], f32r, kind="ExternalInput").ap()
    d_wr = nc.dram_tensor("wf1rT", [D, D], f32r, kind="ExternalInput").ap()
    d_w2 = nc.dram_tensor("wf2T", [D, D], f32r, kind="ExternalInput").ap()
    d_bm = nc.dram_tensor("b_mass", [D, 1], f32, kind="ExternalInput").ap()
    d_b1 = nc.dram_tensor("b_f1", [D, 1], f32, kind="ExternalInput").ap()
    d_b2 = nc.dram_tensor("b_f2", [D, 1], f32, kind="ExternalInput").ap()
    d_mask = nc.dram_tensor("maskdt", [D, NEP], f32, kind="ExternalInput").ap()
    d_ones = nc.dram_tensor("ones1", [1, D], f32r, kind="ExternalInput").ap()
    d_bdec = nc.dram_tensor("bdec2", [NVB, VBLK], f32r, kind="ExternalInput").ap()
    d_wdec = nc.dram_tensor("wdecT", [D, V], f32r, kind="ExternalInput").ap()
    d_ident = nc.dram_tensor("ident", [D, D], f32, kind="ExternalInput").ap()
    d_out = nc.dram_tensor("out", [LOWN, V], f32, kind="ExternalOutput").ap()

    def r(ap):
        return ap.bitcast(f32r)

    with tile.TileContext(nc) as tc:
        with tc.tile_pool(name="const", bufs=1) as const:
            wdec_sb = const.tile([D, V], f32r)
            for i in range(V // WDEC_CHUNK):
                sl = slice(i * WDEC_CHUNK, (i + 1) * WDEC_CHUNK)
                nc.sync.dma_start(wdec_sb[:, sl], d_wdec[:, sl])
            wm_sb = const.tile([D, D], f32r)
            nc.sync.dma_start(wm_sb[:], d_wm[:])
            wl_sb = const.tile([D, D], f32r)
            nc.sync.dma_start(wl_sb[:], d_wl[:])
            wr_sb = const.tile([D, D], f32r)
            nc.sync.dma_start(wr_sb[:], d_wr[:])
            w2_sb = const.tile([D, D], f32r)
            nc.sync.dma_start(w2_sb[:], d_w2[:])
            bm_sb = const.tile([D, 1], f32)
            nc.sync.dma_start(bm_sb[:], d_bm[:])
            b1_sb = const.tile([D, 1], f32)
            nc.sync.dma_start(b1_sb[:], d_b1[:])
            b2_sb = const.tile([D, 1], f32)
            nc.sync.dma_start(b2_sb[:], d_b2[:])
            mask_sb = const.tile([D, NEP], f32)
            nc.sync.dma_start(mask_sb[:], d_mask[:])
            ones_sb = const.tile([1, D], f32r)
            nc.sync.dma_start(ones_sb[:], d_ones[:])
            ident_sb = const.tile([D, D], f32)
            nc.sync.dma_start(ident_sb[:], d_ident[:])

            hT = const.tile([D, LPAD], f32)
            mT = const.tile([D, L + 2], f32)     # 2 finite pad cols feed the fake edge
            fpad = const.tile([D, L + 2], f32)   # col 0 zero border; cols 1..NEP = edges
            dm = const.tile([D, L], f32)

            nc.vector.memset(fpad[:], 0.0)

            # ---- embed gather -> transpose -> hT [D, positions] ----
            with (
                tc.tile_pool(name="gather", bufs=3) as gpool,
                tc.tile_pool(name="idxp", bufs=2) as ipool,
                tc.tile_pool(name="tpsum", bufs=2, space="PSUM") as tpsum,
                tc.tile_pool(name="mpsum", bufs=2, space="PSUM") as mpsum,
            ):
                for t in range(NT):
                    it = ipool.tile([128, 1], i32)
                    nc.sync.dma_start(it[:], d_idx[:, t : t + 1])
                    g = gpool.tile([128, D], f32)
                    nc.gpsimd.indirect_dma_start(
                        out=g[:],
                        out_offset=None,
                        in_=d_emb[:, :],
                        in_offset=bass.IndirectOffsetOnAxis(ap=it[:, :1], axis=0),
                    )
                    pt = tpsum.tile([128, 128], f32)
                    nc.tensor.transpose(pt[:], g[:], ident_sb[:])
                    nc.vector.tensor_copy(r(hT[:, t * 128 : (t + 1) * 128]), pt[:])

                # ---- mass layer: mT = softplus(wmT.T @ hT + b_mass) + EPS ----
                # softplus(z) = ln(exp(z) + 1)  (no native Softplus table on TRN2)
                for off, n in ((0, 512), (512, 512), (1024, L + 2 - 1024)):
                    pm = mpsum.tile([128, 512], f32)
                    nc.tensor.matmul(
                        pm[:, :n], lhsT=wm_sb[:], rhs=r(hT[:, off : off + n]),
                        start=True, stop=True,
                    )
                    nc.scalar.activation(pm[:, :n], pm[:, :n], AF.Exp, bias=bm_sb[:, :1])
                    nc.scalar.activation(r(mT[:, off : off + n]), pm[:, :n], AF.Ln, bias=1.0)
                nc.vector.tensor_scalar_add(r(mT[:, :L]), mT[:, :L], EPS)

            # ---- K flux steps ----
            CHUNKS = ((0, 344), (344, 344), (688, NEP - 688))  # even fp32r free dims
            with (
                tc.tile_pool(name="hid", bufs=3) as hpool,
                tc.tile_pool(name="ps1", bufs=2, space="PSUM") as ps1,
                tc.tile_pool(name="ps2", bufs=2, space="PSUM") as ps2,
            ):
                for _k in range(K):
                    for off, n in CHUNKS:
                        p1 = ps1.tile([128, 344], f32)
                        nc.tensor.matmul(
                            p1[:, :n], lhsT=wl_sb[:], rhs=r(mT[:, off : off + n]),
                            start=True, stop=False,
                        )
                        nc.tensor.matmul(
                            p1[:, :n], lhsT=wr_sb[:], rhs=r(mT[:, off + 1 : off + 1 + n]),
                            start=False, stop=True,
                        )
                        # tanh(z) = 1 - 2/(exp(2z) + 1)
                        nc.scalar.activation(
                            p1[:, :n], p1[:, :n], AF.Exp, bias=b1_sb[:, :1], scale=2.0
                        )
                        nc.vector.tensor_scalar_add(p1[:, :n], p1[:, :n], 1.0)
                        hid = hpool.tile([128, 344], f32)
                        with nc.allow_low_precision(reason="f32r is fp32-width"):
                            nc.vector.reciprocal(r(hid[:, :n]), p1[:, :n])
                        nc.vector.tensor_scalar(
                            r(hid[:, :n]), hid[:, :n], -2.0, 1.0,
                            op0=mybir.AluOpType.mult, op1=mybir.AluOpType.add,
                        )
                        p2 = ps2.tile([128, 344], f32)
                        nc.tensor.matmul(
                            p2[:, :n], lhsT=w2_sb[:], rhs=r(hid[:, :n]),
                            start=True, stop=True,
                        )
                        nc.scalar.activation(p2[:, :n], p2[:, :n], AF.Exp, bias=b2_sb[:, :1])
                        nc.scalar.activation(
                            fpad[:, 1 + off : 1 + off + n], p2[:, :n], AF.Ln, bias=1.0
                        )
                    # F *= mask*dt ; dm = F(left edge) - F(right edge) ; m = max(m+dm, EPS)
                    nc.vector.tensor_mul(fpad[:, 1 : 1 + NEP], fpad[:, 1 : 1 + NEP], mask_sb[:])
                    nc.vector.tensor_sub(dm[:], fpad[:, 0:L], fpad[:, 1 : L + 1])
                    nc.vector.tensor_add(r(mT[:, :L]), mT[:, :L], dm[:])
                    nc.vector.tensor_scalar_max(r(mT[:, :L]), mT[:, :L], EPS)

            # ---- decode: out[r, v] = m_rows @ w_dec.T + b_dec ----
            with (
                tc.tile_pool(name="dpsum", bufs=8, space="PSUM") as dpsum,
                tc.tile_pool(name="stage", bufs=6) as stage,
                tc.tile_pool(name="biasp", bufs=3) as biasp,
            ):
                for v in range(NVB):
                    vs = slice(v * VBLK, (v + 1) * VBLK)
                    bv = biasp.tile([1, VBLK], f32r)
                    nc.sync.dma_start(bv[:], d_bdec[v : v + 1, :])
                    for rt in range(RT):
                        m_blk = mT[:, HALO + rt * 128 : HALO + (rt + 1) * 128]
                        pd = dpsum.tile([128, VBLK], f32)
                        nc.tensor.matmul(
                            pd[:], lhsT=ones_sb[:1, :], rhs=bv[:1, :],
                            start=True, stop=False,
                        )
                        nc.tensor.matmul(
                            pd[:], lhsT=r(m_blk), rhs=wdec_sb[:, vs],
                            start=False, stop=True,
                        )
                        st = stage.tile([128, VBLK], f32)
                        if (v * RT + rt) % 2 == 0:
                            nc.scalar.copy(st[:], pd[:])
                        else:
                            nc.vector.tensor_copy(st[:], pd[:])
                        nc.sync.dma_start(d_out[rt * 128 : (rt + 1) * 128, vs], st[:])

    nc.compile()
    return nc


def _prep_inputs(inputs):
    """Host-side shard prep: per-core input dict list."""
    x = np.asarray(inputs["x"]).astype(np.int32)            # [B, S]
    emb = np.ascontiguousarray(np.asarray(inputs["emb"], np.float32))
    w_mass = np.asarray(inputs["w_mass"], np.float32)
    b_mass = np.asarray(inputs["b_mass"], np.float32)
    w_f1 = np.asarray(inputs["w_f1"], np.float32)
    b_f1 = np.asarray(inputs["b_f1"], np.float32)
    w_f2 = np.asarray(inputs["w_f2"], np.float32)
    b_f2 = np.asarray(inputs["b_f2"], np.float32)
    cfl = float(np.asarray(inputs["cfl_raw"]))
    w_dec = np.asarray(inputs["w_dec"], np.float32)
    b_dec = np.asarray(inputs["b_dec"], np.float32)

    dt = float(1.0 / (1.0 + np.exp(-cfl)))

    common = {
        "emb": emb,
        "wmT": np.ascontiguousarray(w_mass.T),
        "wf1lT": np.ascontiguousarray(w_f1[:, :D].T),
        "wf1rT": np.ascontiguousarray(w_f1[:, D:].T),
        "wf2T": np.ascontiguousarray(w_f2.T),
        "b_mass": np.ascontiguousarray(b_mass[:, None]),
        # device computes tanh(z+b) as 1 - 2/(exp(2z + 2b) + 1) with scale=2 on z
        "b_f1": np.ascontiguousarray((2.0 * b_f1)[:, None]),
        "b_f2": np.ascontiguousarray(b_f2[:, None]),
        "ones1": np.ones((1, D), np.float32),
        "bdec2": np.ascontiguousarray(b_dec.reshape(NVB, VBLK)),
        "wdecT": np.ascontiguousarray(w_dec.T),
        "ident": np.eye(D, dtype=np.float32),
    }

    in_maps = []
    for c in range(NCORES):
        b, half = divmod(c, 2)
        idx = np.zeros(LPAD, np.int32)
        mask = np.full(NEP, dt, np.float32)
        if half == 0:
            idx[HALO : HALO + (L - HALO)] = x[b, 0 : L - HALO]
            mask[0:HALO] = 0.0
        else:
            idx[0 : L - HALO] = x[b, S - (L - HALO) : S]
            mask[NE - HALO : NE] = 0.0
        m = dict(common)
        m["idx"] = np.ascontiguousarray(idx.reshape(NT, 128).T)     # [128, NT]
        m["maskdt"] = np.ascontiguousarray(
            np.broadcast_to(mask[None, :], (D, NEP)).astype(np.float32)
        )
        in_maps.append(m)
    return in_maps


def get_program():
    if "nc" not in _CACHE:
        _CACHE["nc"] = build_program()
    return _CACHE["nc"]


def run(inputs, trace=False, **kw):
    """Returns (full_output [B,S,V] float32, BassKernelResults)."""
    from concourse.bass_utils import run_bass_kernel_spmd

    nc = get_program()
    in_maps = _prep_inputs(inputs)
    res = run_bass_kernel_spmd(
        nc, in_maps, core_ids=list(range(NCORES)), trace=trace, **kw
    )
    full = np.empty((B * S, V), np.float32)
    for c in range(NCORES):
        full[c * LOWN : (c + 1) * LOWN] = res.results[c]["out"]
    return full.reshape(B, S, V), res


def kernel(**inputs):
    out, _ = run(inputs, trace=False)
    return out


# revision 19
# speedup vs baseline: 56.0553x; 56.0553x over previous
"""Trainium2 Bass kernel for nn_DTFN_38405597561803 (gnn_message_passing).

Model (reference):
    h  = emb[x]                                   # [B,S,D] gather
    m  = softplus(h @ w_mass.T + b_mass) + EPS
    dt = sigmoid(cfl_raw)
    repeat K=3:
        hid = tanh(left @ w_f1_l.T + right @ w_f1_r.T + b_f1)   # left/right = adjacent positions
        F   = softplus(hid @ w_f2.T + b_f2)
        m   = max(m + dt * (F[i-1] - F[i]), EPS)                # conservative 1-D flux stencil
    out = m @ w_dec.T + b_dec                      # [B,S,V] decode, V=32000 (memory bound)

Sharding: sequence-parallel, 8 cores = 4 batches x 2 halves of S=2048.
Each core gets its 1024 owned positions plus a K=3 halo on each interior
side (built host-side by overlapping the shards -> no device-to-device
communication).  Chunk boundaries that are not true sequence boundaries
are handled exactly by masking the 3 outermost edge fluxes each step
(errors cannot cross a masked edge); true boundaries coincide with chunk
ends so the no-flux boundary condition is automatic.  dt is folded into
the edge mask.  Layout on device is feature-major (mT = [D, positions]) so
the stencil shift is a free-dim AP offset and all matmuls need no
transposes (only the initial embedding gather is transposed via the PE).

Decode: out[r, v] tiles of [128 rows, 500 vocab]; stationary = mT column
block, moving = resident w_dec.T slice (float32r matmuls, full PE rate);
b_dec added by an extra K=1 matmul with a ones vector accumulating into
the same PSUM bank; PSUM -> SBUF copies alternate Scalar/Vector engines;
SBUF -> HBM DMA writes 131 MB/core (the memory-roofline term).
"""

import sys

if "/opt/trn_rl_repo" not in sys.path:
    sys.path.insert(0, "/opt/trn_rl_repo")

import numpy as np

B, S, D, V, K = 4, 2048, 128, 32000, 3
EPS = 1e-6
NCORES = 8
HALO = K                      # 3
LOWN = S // 2                 # 1024 owned positions per core
L = LOWN + 2 * HALO           # 1030 local positions
NE = L - 1                    # 1029 local edges
NEP = NE + 1                  # 1030: padded even edge count (fp32r needs even free dims);
                              # the extra fake edge only ever pollutes the outermost halo
                              # position (1 col/step, 3 steps, owned ends 3 cols earlier)
NT = (L + 127) // 128         # 9 gather tiles (covers 1152 >= 1030)
LPAD = NT * 128               # 1152
VBLK = 500                    # vocab block (<= 512 PSUM bank, 64*500 = 32000)
NVB = V // VBLK               # 64
RT = LOWN // 128              # 8 row tiles per core
WDEC_CHUNK = 500              # w_decT load granularity (64 x 250KB; small so prologue gathers aren't blocked behind big transfers)

_CACHE = {}


def build_program(decode_reps=1):
    """Build (and bacc-compile) the single-core SPMD Bass program.

    decode_reps > 1 wraps the decode phase in a hardware For_i loop that
    repeats it (same output written each pass) — benchmarking only, to
    measure the steady-state decode pass time by slope.
    """
    import concourse.bacc as bacc
    import concourse.bass as bass
    import concourse.tile as tile
    from concourse import mybir

    f32 = mybir.dt.float32
    f32r = mybir.dt.float32r
    i32 = mybir.dt.int32
    AF = mybir.ActivationFunctionType

    nc = bacc.Bacc(
        trn_type="TRN2",
        target_bir_lowering=False,
        debug=False,
        enable_asserts=False,
        num_devices=NCORES,
    )

    d_idx = nc.dram_tensor("idx", [128, NT], i32, kind="ExternalInput").ap()
    d_emb = nc.dram_tensor("emb", [V, D], f32, kind="ExternalInput").ap()
    d_wm = nc.dram_tensor("wmT", [D, D], f32r, kind="ExternalInput").ap()
    d_wl = nc.dram_tensor("wf1lT", [D, D], f32r, kind="ExternalInput").ap()
    d_wr = nc.dram_tensor("wf1rT", [D, D], f32r, kind="ExternalInput").ap()
    d_w2 = nc.dram_tensor("wf2T", [D, D], f32r, kind="ExternalInput").ap()
    d_bm = nc.dram_tensor("b_mass", [D, 1], f32, kind="ExternalInput").ap()
    d_b1 = nc.dram_tensor("b_f1", [D, 1], f32, kind="ExternalInput").ap()
    d_b2 = nc.dram_tensor("b_f2", [D, 1], f32, kind="ExternalInput").ap()
    d_mask = nc.dram_tensor("maskdt", [D, NEP], f32, kind="ExternalInput").ap()
    d_ones = nc.dram_tensor("ones1", [1, D], f32r, kind="ExternalInput").ap()
    d_bdec = nc.dram_tensor("bdec2", [NVB, VBLK], f32r, kind="ExternalInput").ap()
    d_wdec = nc.dram_tensor("wdecT", [D, V], f32r, kind="ExternalInput").ap()
    d_ident = nc.dram_tensor("ident", [D, D], f32, kind="ExternalInput").ap()
    d_out = nc.dram_tensor("out", [LOWN, V], f32, kind="ExternalOutput").ap()

    def r(ap):
        return ap.bitcast(f32r)

    with tile.TileContext(nc) as tc:
        with tc.tile_pool(name="const", bufs=1) as const:
            wdec_sb = const.tile([D, V], f32r)
            wm_sb = const.tile([D, D], f32r)
            nc.scalar.dma_start(wm_sb[:], d_wm[:])
            wl_sb = const.tile([D, D], f32r)
            nc.scalar.dma_start(wl_sb[:], d_wl[:])
            wr_sb = const.tile([D, D], f32r)
            nc.scalar.dma_start(wr_sb[:], d_wr[:])
            w2_sb = const.tile([D, D], f32r)
            nc.scalar.dma_start(w2_sb[:], d_w2[:])
            bm_sb = const.tile([D, 1], f32)
            nc.scalar.dma_start(bm_sb[:], d_bm[:])
            b1_sb = const.tile([D, 1], f32)
            nc.scalar.dma_start(b1_sb[:], d_b1[:])
            b2_sb = const.tile([D, 1], f32)
            nc.scalar.dma_start(b2_sb[:], d_b2[:])
            mask_sb = const.tile([D, NEP], f32)
            nc.scalar.dma_start(mask_sb[:], d_mask[:])
            ones_sb = const.tile([1, D], f32r)
            nc.scalar.dma_start(ones_sb[:], d_ones[:])
            ident_sb = const.tile([D, D], f32)
            nc.scalar.dma_start(ident_sb[:], d_ident[:])

            hT = const.tile([D, LPAD], f32)
            mT = const.tile([D, L + 2], f32)     # 2 finite pad cols feed the fake edge
            fpad = const.tile([D, L + 2], f32)   # col 0 zero border; cols 1..NEP = edges
            dm = const.tile([D, L], f32)

            nc.vector.memset(fpad[:], 0.0)

            # ---- embed gather -> transpose -> hT [D, positions] ----
            with (
                tc.tile_pool(name="gather", bufs=3) as gpool,
                tc.tile_pool(name="idxp", bufs=2) as ipool,
                tc.tile_pool(name="tpsum", bufs=2, space="PSUM") as tpsum,
                tc.tile_pool(name="mpsum", bufs=2, space="PSUM") as mpsum,
            ):
                for t in range(NT):
                    it = ipool.tile([128, 1], i32)
                    nc.scalar.dma_start(it[:], d_idx[:, t : t + 1])
                    g = gpool.tile([128, D], f32)
                    nc.gpsimd.indirect_dma_start(
                        out=g[:],
                        out_offset=None,
                        in_=d_emb[:, :],
                        in_offset=bass.IndirectOffsetOnAxis(ap=it[:, :1], axis=0),
                    )
                    pt = tpsum.tile([128, 128], f32)
                    nc.tensor.transpose(pt[:], g[:], ident_sb[:])
                    nc.vector.tensor_copy(r(hT[:, t * 128 : (t + 1) * 128]), pt[:])

                # ---- mass layer: mT = softplus(wmT.T @ hT + b_mass) + EPS ----
                # softplus(z) = ln(exp(z) + 1)  (no native Softplus table on TRN2)
                for off, n in ((0, 512), (512, 512), (1024, L + 2 - 1024)):
                    pm = mpsum.tile([128, 512], f32)
                    nc.tensor.matmul(
                        pm[:, :n], lhsT=wm_sb[:], rhs=r(hT[:, off : off + n]),
                        start=True, stop=True,
                    )
                    nc.scalar.activation(pm[:, :n], pm[:, :n], AF.Exp, bias=bm_sb[:, :1])
                    nc.scalar.activation(r(mT[:, off : off + n]), pm[:, :n], AF.Ln, bias=1.0)
                nc.vector.tensor_scalar_add(r(mT[:, :L]), mT[:, :L], EPS)

            # ---- K flux steps ----
            CHUNKS = ((0, 512), (512, 512), (1024, NEP - 1024))  # even fp32r free dims
            with (
                tc.tile_pool(name="hid", bufs=3) as hpool,
                tc.tile_pool(name="ps1", bufs=2, space="PSUM") as ps1,
                tc.tile_pool(name="ps2", bufs=2, space="PSUM") as ps2,
            ):
                for _k in range(K):
                    for off, n in CHUNKS:
                        p1 = ps1.tile([128, 512], f32)
                        nc.tensor.matmul(
                            p1[:, :n], lhsT=wl_sb[:], rhs=r(mT[:, off : off + n]),
                            start=True, stop=False,
                        )
                        nc.tensor.matmul(
                            p1[:, :n], lhsT=wr_sb[:], rhs=r(mT[:, off + 1 : off + 1 + n]),
                            start=False, stop=True,
                        )
                        # tanh(z) = 1 - 2/(exp(2z) + 1)
                        nc.scalar.activation(
                            p1[:, :n], p1[:, :n], AF.Exp, bias=b1_sb[:, :1], scale=2.0
                        )
                        nc.vector.tensor_scalar_add(p1[:, :n], p1[:, :n], 1.0)
                        hid = hpool.tile([128, 512], f32)
                        with nc.allow_low_precision(reason="f32r is fp32-width"):
                            nc.vector.reciprocal(r(hid[:, :n]), p1[:, :n])
                        nc.vector.tensor_scalar(
                            r(hid[:, :n]), hid[:, :n], -2.0, 1.0,
                            op0=mybir.AluOpType.mult, op1=mybir.AluOpType.add,
                        )
                        p2 = ps2.tile([128, 512], f32)
                        nc.tensor.matmul(
                            p2[:, :n], lhsT=w2_sb[:], rhs=r(hid[:, :n]),
                            start=True, stop=True,
                        )
                        nc.scalar.activation(p2[:, :n], p2[:, :n], AF.Exp, bias=b2_sb[:, :1])
                        nc.scalar.activation(
                            fpad[:, 1 + off : 1 + off + n], p2[:, :n], AF.Ln, bias=1.0
                        )
                    # F *= mask*dt ; dm = F(left edge) - F(right edge) ; m = max(m+dm, EPS)
                    nc.vector.tensor_mul(fpad[:, 1 : 1 + NEP], fpad[:, 1 : 1 + NEP], mask_sb[:])
                    nc.vector.tensor_sub(dm[:], fpad[:, 0:L], fpad[:, 1 : L + 1])
                    nc.vector.tensor_add(r(mT[:, :L]), mT[:, :L], dm[:])
                    nc.vector.tensor_scalar_max(r(mT[:, :L]), mT[:, :L], EPS)

            # w_dec.T loads issued after the stencil in program order so the
            # Tile scheduler runs them on DMA idle time during the (serial)
            # stencil instead of ahead of the latency-critical gather chain.
            for i in range(V // WDEC_CHUNK):
                sl = slice(i * WDEC_CHUNK, (i + 1) * WDEC_CHUNK)
                nc.sync.dma_start(wdec_sb[:, sl], d_wdec[:, sl])

            # ---- decode: out[r, v] = m_rows @ w_dec.T + b_dec ----
            from contextlib import nullcontext
            with (
                tc.tile_pool(name="dpsum", bufs=8, space="PSUM") as dpsum,
                tc.tile_pool(name="stage", bufs=12) as stage,
                tc.tile_pool(name="biasp", bufs=4) as biasp,
                tc.For_i(
                    0, decode_reps, 1,
                    hint_engines=(
                        mybir.EngineType.PE, mybir.EngineType.Activation,
                        mybir.EngineType.DVE, mybir.EngineType.SP,
                        mybir.EngineType.Pool,
                    ),
                ) if decode_reps > 1 else nullcontext(),
            ):
                for v in range(NVB):
                    vs = slice(v * VBLK, (v + 1) * VBLK)
                    bv = biasp.tile([1, VBLK], f32r)
                    nc.gpsimd.dma_start(bv[:], d_bdec[v : v + 1, :])
                    for rt in range(RT):
                        i = v * RT + rt
                        m_blk = mT[:, HALO + rt * 128 : HALO + (rt + 1) * 128]
                        pd = dpsum.tile([128, VBLK], f32)
                        nc.tensor.matmul(
                            pd[:], lhsT=ones_sb[:1, :], rhs=bv[:1, :],
                            start=True, stop=False,
                        )
                        nc.tensor.matmul(
                            pd[:], lhsT=r(m_blk), rhs=wdec_sb[:, vs],
                            start=False, stop=True,
                        )
                        st = stage.tile([128, VBLK], f32)
                        if i % 2 == 0:
                            nc.scalar.copy(st[:], pd[:])
                        else:
                            nc.vector.tensor_copy(st[:], pd[:])
                        # alternate the two HWDGE rings (SP / ACT) for the
                        # 131 MB output stream
                        dma_eng = nc.sync if i % 2 == 0 else nc.scalar
                        dma_eng.dma_start(d_out[rt * 128 : (rt + 1) * 128, vs], st[:])

    # All ACT functions used here (Exp, Ln, Copy) live in the single
    # 'natural_log_exp_and_others' table, but the table-load insertion pass
    # picks the first table containing each function, thrashing
    # LoadActFuncSet (~1.3us each) between exp_and_others / natural_log.
    # Blank out every other table (positions preserved so act_func_set_id
    # still indexes act_info.json correctly) so one load serves the kernel.
    import concourse.bacc as bacc_mod
    orig_get_tables = bacc_mod.get_activation_tables

    def only_ln_exp(arch):
        tabs = orig_get_tables(arch)
        return {
            k: (v if k == "natural_log_exp_and_others" else set())
            for k, v in tabs.items()
        }

    bacc_mod.get_activation_tables = only_ln_exp
    try:
        nc.compile()
    finally:
        bacc_mod.get_activation_tables = orig_get_tables
    return nc


def _prep_inputs(inputs):
    """Host-side shard prep: per-core input dict list."""
    x = np.asarray(inputs["x"]).astype(np.int32)            # [B, S]
    emb = np.ascontiguousarray(np.asarray(inputs["emb"], np.float32))
    w_mass = np.asarray(inputs["w_mass"], np.float32)
    b_mass = np.asarray(inputs["b_mass"], np.float32)
    w_f1 = np.asarray(inputs["w_f1"], np.float32)
    b_f1 = np.asarray(inputs["b_f1"], np.float32)
    w_f2 = np.asarray(inputs["w_f2"], np.float32)
    b_f2 = np.asarray(inputs["b_f2"], np.float32)
    cfl = float(np.asarray(inputs["cfl_raw"]))
    w_dec = np.asarray(inputs["w_dec"], np.float32)
    b_dec = np.asarray(inputs["b_dec"], np.float32)

    dt = float(1.0 / (1.0 + np.exp(-cfl)))

    common = {
        "emb": emb,
        "wmT": np.ascontiguousarray(w_mass.T),
        "wf1lT": np.ascontiguousarray(w_f1[:, :D].T),
        "wf1rT": np.ascontiguousarray(w_f1[:, D:].T),
        "wf2T": np.ascontiguousarray(w_f2.T),
        "b_mass": np.ascontiguousarray(b_mass[:, None]),
        # device computes tanh(z+b) as 1 - 2/(exp(2z + 2b) + 1) with scale=2 on z
        "b_f1": np.ascontiguousarray((2.0 * b_f1)[:, None]),
        "b_f2": np.ascontiguousarray(b_f2[:, None]),
        "ones1": np.ones((1, D), np.float32),
        "bdec2": np.ascontiguousarray(b_dec.reshape(NVB, VBLK)),
        "wdecT": np.ascontiguousarray(w_dec.T),
        "ident": np.eye(D, dtype=np.float32),
    }

    in_maps = []
    for c in range(NCORES):
        b, half = divmod(c, 2)
        idx = np.zeros(LPAD, np.int32)
        mask = np.full(NEP, dt, np.float32)
        if half == 0:
            idx[HALO : HALO + (L - HALO)] = x[b, 0 : L - HALO]
            mask[0:HALO] = 0.0
        else:
            idx[0 : L - HALO] = x[b, S - (L - HALO) : S]
            mask[NE - HALO : NE] = 0.0
        m = dict(common)
        m["idx"] = np.ascontiguousarray(idx.reshape(NT, 128).T)     # [128, NT]
        m["maskdt"] = np.ascontiguousarray(
            np.broadcast_to(mask[None, :], (D, NEP)).astype(np.float32)
        )
        in_maps.append(m)
    return in_maps


def get_program():
    if "nc" not in _CACHE:
        _CACHE["nc"] = build_program()
    return _CACHE["nc"]


def run(inputs, trace=False, **kw):
    """Returns (full_output [B,S,V] float32, BassKernelResults)."""
    from concourse.bass_utils import run_bass_kernel_spmd

    nc = get_program()
    in_maps = _prep_inputs(inputs)
    res = run_bass_kernel_spmd(
        nc, in_maps, core_ids=list(range(NCORES)), trace=trace, **kw
    )
    full = np.empty((B * S, V), np.float32)
    for c in range(NCORES):
        full[c * LOWN : (c + 1) * LOWN] = res.results[c]["out"]
    return full.reshape(B, S, V), res


def kernel(**inputs):
    out, _ = run(inputs, trace=False)
    return out
